# revision 61
# baseline (speedup 1.0000x reference)
"""Trainium2 Bass kernel for nn_Attention_49074296324413.

Data-parallel over batch: core b handles batch element b of
  kv = dw3x3(conv1x1(x, w_kv)); k, v = split(kv); k = avgpool2x2(k)
  q  = conv3x3(conv1x1(y, w_q))
  out = conv1x1(softmax(norm(q) @ norm(k).T * temp) @ v, w_proj)

Structure (156.5us baseline -> ~136us):
 - q path: the 1x1 conv is folded into the full 3x3 conv host-side
   (W2[co,ci,t] = sum_m wqdw[co,m,t] wq[m,ci], exact) and the fused conv
   runs FIRST: it needs only ~1.9MB of DMA (y3p host-padded + one
   co-major w2T chunk), so the PE computes while the 4.5MB x streams
   land.  ~8 warm-up matmuls before it flip the HAM clock-gate
   (PE idles cold at 1.2GHz otherwise) during the DMA-issue window.
 - q/k path entirely fp8(e4m3) with DoubleRow (2 contraction blocks per
   pass; pair strides kept %16): quantization noise is washed by the
   per-channel L2 norms + softmax.  k depthwise+pool is folded to a
   4x4-stride-2 conv run as diagonal DR matmuls (8 passes for 16 taps).
 - v path stays bf16 (its error passes straight to the output).
   v-depthwise: 6 taps as PE diag matmuls; taps (1,0),(1,1),(1,2) on
   scalar+vector, folded into the PSUM evacuation (the Tile scheduler
   hoists these builds into earlier idle slots).
 - attn@v and the projection are fused: out = Mst.T @ v_dw with
   M stacked per head from attn_h.T @ w_proj slices.
 - QK/M run on head PAIRS in a head-padded transposed layout
   ([128, 8 heads, 64] with heads at 64-col slots so pair blocks sit at
   32-aligned partitions): 32 QK matmuls + 4 pair M matmuls.
 - PSUM tiles are [128,1024] with single merged evacuations (PSUM f32
   reads run at 1x on the DVE, so fewer/larger evacs matter); transposes
   evacuate 3-at-a-time as [128,384] bf16; the final output group splits
   its evac+DMA across engines to shorten the tail.

Run-to-run variance note: the chip drops the PE to ~2.0GHz under
sustained power draw (P0), which adds ~15-20us on unlucky runs.
"""
import numpy as np
import ml_dtypes

import concourse.bass as bass
import concourse.tile as tile
from concourse import bacc, mybir
from concourse.ap import AP
from concourse.bass_utils import run_bass_kernel_spmd

dt = mybir.dt
BF = dt.bfloat16
F8 = dt.float8e4
F32 = dt.float32
AF = mybir.ActivationFunctionType
OP = mybir.AluOpType
DR = mybir.MatmulPerfMode.DoubleRow

DIM = 384
HEADS = 8
HC = DIM // HEADS          # 48 channels per head
CT = DIM // 128            # 3 channel tiles
H = 64                     # x spatial
NPIX = H * H               # 4096
H2 = 32                    # y spatial
NPIX2 = H2 * H2            # 1024
PW = 66                    # padded rows for 64-grid
RW = 80                    # padded row pitch (elems) for 64-grid
PW2 = 34                   # padded rows for 32-grid
RW2 = 48                   # padded row pitch for 32-grid
CB = PW * RW               # 5280 elems per channel-tile block (64-grid)
CB2 = PW2 * RW2            # 1632 elems per channel-tile block (32-grid)

BF_NP = ml_dtypes.bfloat16
F8_NP = ml_dtypes.float8_e4m3

S_A = 2.0 ** 6     # w_k scale (washed by k-norm)
S_DW = 2.0 ** 11   # w4k (k depthwise+pool) scale (washed by k-norm)


def _ap(base: AP, off: int, dims):
    """Custom strided AP into a tile's free space: dims = [[stride, n], ...]."""
    return AP(tensor=base.tensor, offset=base.offset + off,
              ap=[[base.ap[0][0], base.ap[0][1]]] + [list(d) for d in dims])


def build_program(dbg: bool = False):
    nc = bacc.Bacc("TRN2", target_bir_lowering=False, debug=False)

    xb_d = nc.dram_tensor("xb3", (128, CT * NPIX), BF, kind="ExternalInput")
    x8_d = nc.dram_tensor("x8", (128, CT * NPIX), F8, kind="ExternalInput")
    y3p_d = nc.dram_tensor("y3p", (128, CT * CB2), F8, kind="ExternalInput")
    wkT_d = nc.dram_tensor("wkT3", (128, CT * DIM), F8, kind="ExternalInput")
    wvT_d = nc.dram_tensor("wvT3", (128, CT * DIM), BF, kind="ExternalInput")
    w2T_d = nc.dram_tensor("w2T", (128, CT * 9 * DIM), F8, kind="ExternalInput")
    w3v_d = nc.dram_tensor("w3vc", (128, CT * 9), F32, kind="ExternalInput")
    dgv_d = nc.dram_tensor("dgv", (128, CT * 9 * 128), BF, kind="ExternalInput")
    dgk_d = nc.dram_tensor("dgk", (128, CT * 2048), F8, kind="ExternalInput")
    wpT_d = nc.dram_tensor("wpP", (128, 4 * DIM), BF, kind="ExternalInput")
    temp_d = nc.dram_tensor("tempc", (128, CT), F32, kind="ExternalInput")
    idn_d = nc.dram_tensor("idn", (128, 128), BF, kind="ExternalInput")

    out_d = nc.dram_tensor("out", (DIM, NPIX), BF, kind="ExternalOutput")
    dbg_d = {}
    if dbg:
        dbg_d["vdw"] = nc.dram_tensor("dbg_vdw", (DIM, NPIX), BF, kind="ExternalOutput")
        dbg_d["kpn"] = nc.dram_tensor("dbg_kpn", (DIM, NPIX2), BF, kind="ExternalOutput")
        dbg_d["q3n"] = nc.dram_tensor("dbg_q3n", (DIM, NPIX2), BF, kind="ExternalOutput")
        dbg_d["att"] = nc.dram_tensor("dbg_att", (HEADS * HC, HC), BF, kind="ExternalOutput")
        dbg_d["mst"] = nc.dram_tensor("dbg_mst", (128, CT * DIM), BF, kind="ExternalOutput")

    with tile.TileContext(nc) as tc:
        _emit(nc, tc, xb_d, x8_d, y3p_d, wkT_d, wvT_d, w2T_d, w3v_d,
              dgv_d, dgk_d, wpT_d, temp_d, idn_d, out_d, dbg_d)
    nc.compile()
    return nc


def _emit(nc, tc, xb_d, x8_d, y3p_d, wkT_d, wvT_d, w2T_d, w3v_d,
          dgv_d, dgk_d, wpT_d, temp_d, idn_d, out_d, dbg_d):
    from contextlib import ExitStack
    ctx = ExitStack()

    cst = ctx.enter_context(tc.tile_pool(name="cst", bufs=1))
    big = ctx.enter_context(tc.tile_pool(name="big", bufs=1))
    wrk = ctx.enter_context(tc.tile_pool(name="wrk", bufs=2))
    osb = ctx.enter_context(tc.tile_pool(name="osb", bufs=4))
    ps_big = ctx.enter_context(tc.tile_pool(name="ps_big", bufs=2, space="PSUM"))
    ps_sm = ctx.enter_context(tc.tile_pool(name="ps_sm", bufs=2, space="PSUM"))
    ps_tr = ctx.enter_context(tc.tile_pool(name="ps_tr", bufs=2, space="PSUM"))

    # ---------------- PE warm-up (HAM un-throttle during DMA window) ------
    warm = cst.tile([128, 512], BF, tag="warm", name="warm")
    nc.vector.memset(warm[:], 0.25)

    # ---------------- input DMAs, priority-ordered per queue --------------
    # D (fused q conv) runs FIRST while the big xb/x8 streams land, so the
    # sync queue leads with w2T co-chunks and scalar with y3p.
    w2T_t = cst.tile([128, CT * 9 * DIM], F8, tag="w2T", name="w2T")
    w2v = w2T_t[:].rearrange("p (a b) -> p a b", a=CT)
    w2d = w2T_d.ap().rearrange("p (a b) -> p a b", a=CT)
    nc.sync.dma_start(w2v[:, 0], w2d[:, 0])
    y3p_t = cst.tile([128, CT, PW2, RW2], F8, tag="y3p", name="y3p")
    nc.scalar.dma_start(y3p_t[:].rearrange("p a b c -> p (a b c)"), y3p_d.ap())
    xb_t = cst.tile([128, CT, NPIX], BF, tag="xb3", name="xb3")
    xbv = xb_d.ap().rearrange("p (a b) -> p a b", a=CT)
    # hold gpsimd's first (late-needed) xb issue until w2T chunk 0 has
    # landed, so D's critical-path DMA gets the full HBM bandwidth
    gate = cst.tile([128, 1], F8, tag="gate", name="gate")
    nc.gpsimd.tensor_copy(gate[:], w2T_t[:, 0:1])
    nc.gpsimd.dma_start(xb_t[:, :, 1024:2048], xbv[:, :, 1024:2048])
    nc.sync.dma_start(w2v[:, 1], w2d[:, 1])
    nc.sync.dma_start(w2v[:, 2], w2d[:, 2])
    dgv_t = cst.tile([128, CT, 9, 128], BF, tag="dgv", name="dgv")
    nc.sync.dma_start(dgv_t[:].rearrange("p a b c -> p (a b c)"), dgv_d.ap())
    tempc_t = cst.tile([128, CT], F32, tag="tempc", name="tempc")
    nc.scalar.dma_start(tempc_t[:], temp_d.ap())
    wvT_t = cst.tile([128, CT * DIM], BF, tag="wvT", name="wvT")
    nc.scalar.dma_start(wvT_t[:], wvT_d.ap())
    nc.sync.dma_start(xb_t[:, :, 0:1024], xbv[:, :, 0:1024])
    nc.gpsimd.dma_start(xb_t[:, :, 3072:4096], xbv[:, :, 3072:4096])
    wkT_t = cst.tile([128, CT * DIM], F8, tag="wkT", name="wkT")
    nc.scalar.dma_start(wkT_t[:], wkT_d.ap())
    nc.sync.dma_start(xb_t[:, :, 2048:3072], xbv[:, :, 2048:3072])
    w3v_t = cst.tile([128, CT * 9], F32, tag="w3vc", name="w3vc")
    nc.scalar.dma_start(w3v_t[:], w3v_d.ap())
    dgk_t = cst.tile([128, CT * 2048], F8, tag="dgk", name="dgk")
    nc.scalar.dma_start(dgk_t[:], dgk_d.ap())
    idn_t = cst.tile([128, 128], BF, tag="idn", name="idn")
    nc.scalar.dma_start(idn_t[:], idn_d.ap())
    wpT_t = cst.tile([128, 4, DIM], BF, tag="wpP", name="wpP")
    nc.scalar.dma_start(wpT_t[:].rearrange("p a b -> p (a b)"), wpT_d.ap())

    eps_col = cst.tile([128, 1], F32, tag="eps_col", name="eps_col")
    nc.vector.memset(eps_col[:], 1e-24)
    zero_col = cst.tile([128, 1], F32, tag="zero_col", name="zero_col")
    nc.vector.memset(zero_col[:], 0.0)

    # warm-up matmuls at the cold clock: flips the HAM SHORT window and
    # fills the PE until D's first DMA dependencies land (~13us).
    for _ in range(10):
        psw = ps_sm.tile([128, 512], F32, tag="ps_sm", name="ps_sm")
        nc.tensor.matmul(psw[:], warm[:, 0:128], warm[:], start=True, stop=True)

    # ---------------- padded image buffers (zero borders) ----------------
    kpad = big.tile([128, CT, PW, RW], F8, tag="kpad", name="kpad")
    vpad = big.tile([128, CT, PW, RW], BF, tag="vpad", name="vpad")
    for ct in range(CT):
        for t, pw in ((kpad, PW), (vpad, PW)):
            eng = nc.vector if ct % 2 == 0 else nc.gpsimd
            eng.memset(t[:, ct, 0, :], 0.0)
            eng.memset(t[:, ct, pw - 1, :], 0.0)
            eng.memset(t[:, ct, 1:pw - 1, 0:1], 0.0)
            eng.memset(t[:, ct, 1:pw - 1, pw - 1:pw], 0.0)
    kpadf = kpad[:].rearrange("p a b c -> p (a b c)")
    vpadf = vpad[:].rearrange("p a b c -> p (a b c)")
    y3pf = y3p_t[:].rearrange("p a b c -> p (a b c)")

    # ---------------- x8 = fp8(x) via DMA (behind the xb chunks) ----------
    x8_t = cst.tile([128, CT, NPIX], F8, tag="x8", name="x8")
    x8f = x8_t[:].rearrange("p a b -> p (a b)")
    x8v = x8_d.ap().rearrange("p (a b) -> p a b", a=CT)
    nc.gpsimd.dma_start(x8_t[:, :, 0:2048], x8v[:, :, 0:2048])
    nc.gpsimd.dma_start(x8_t[:, :, 2048:4096], x8v[:, :, 2048:4096])

    # ---------------- phase D: q3 = fused conv3x3(conv1x1(y)) (fp8 DR) ----
    # Runs FIRST: needs only y3p + one w2T co-chunk, filling the PE while
    # the xb/x8 streams land.  Contraction blocks b=(ci,dy) lex-ordered;
    # pairs share dx.  E (q norm + temperature) follows per co.
    q3_t = [big.tile([128, NPIX2], BF, tag=f"q3{ct}", name=f"q3{ct}") for ct in range(CT)]
    blocks = [(ci, dy) for ci in range(CT) for dy in range(3)]
    for co in range(CT):
        ps = ps_big.tile([128, 1024], F32, tag="ps_big", name="ps_big")
        for j in range(2):
            for dx in range(3):
                for p in range(4):
                    ci0, dy0 = blocks[2 * p]
                    ci1, dy1 = blocks[2 * p + 1]
                    m0 = ci0 * CB2 + (16 * j + dy0) * RW2 + dx
                    dm = (ci1 - ci0) * CB2 + (dy1 - dy0) * RW2
                    w0 = co * 9 * DIM + (ci0 * 9 + 3 * dy0 + dx) * 128
                    nc.tensor.matmul(
                        ps[:, 512 * j:512 * (j + 1)],
                        _ap(w2T_t[:], w0, [[3 * 128, 2], [1, 128]]),
                        _ap(y3pf, m0, [[dm, 2], [RW2, 16], [1, 32]]),
                        start=(dx == 0 and p == 0), stop=False, perf_mode=DR)
                m8 = 2 * CB2 + (16 * j + 2) * RW2 + dx
                w8 = co * 9 * DIM + (2 * 9 + 6 + dx) * 128
                nc.tensor.matmul(
                    ps[:, 512 * j:512 * (j + 1)],
                    _ap(w2T_t[:], w8, [[1, 128]]),
                    _ap(y3pf, m8, [[RW2, 16], [1, 32]]),
                    start=False, stop=(dx == 2))
        # phase E: q norm + temperature for this co, fused with the PSUM
        # evacuation (Square reads PSUM; the normalize mul writes q3_t).
        sq = wrk.tile([128, NPIX2], BF, tag="sqq", name="sqq")
        nrm2 = wrk.tile([128, 1], F32, tag="nrm2q", name="nrm2q")
        nc.scalar.activation(sq[:], ps[:], AF.Square, bias=zero_col[:],
                             accum_out=nrm2[:])
        nrm = wrk.tile([128, 1], F32, tag="nrmq", name="nrmq")
        nc.scalar.activation(nrm[:], nrm2[:], AF.Sqrt, bias=eps_col[:])
        inv = wrk.tile([128, 1], F32, tag="invq", name="invq")
        nc.vector.reciprocal(inv[:], nrm[:])
        invt = wrk.tile([128, 1], F32, tag="invqt", name="invqt")
        nc.scalar.mul(invt[:], inv[:], tempc_t[:, co:co + 1])
        nc.vector.tensor_scalar_mul(q3_t[co][:], ps[:], invt[:])
        if "q3n" in dbg_d:
            nc.sync.dma_start(dbg_d["q3n"].ap()[128 * co:128 * (co + 1), :], q3_t[co][:])

    # ---------------- phase A: kv1 = W_kv @ x -----------------------------
    # v half bf16 (precision), k half fp8 DR; FD=1024.
    def a_block(co, c):
        ps = ps_big.tile([128, 1024], F32, tag="ps_big", name="ps_big")
        if co >= CT:     # v half
            ct = co - CT
            for j in range(2):
                for ci in range(CT):
                    nc.tensor.matmul(
                        ps[:, 512 * j:512 * (j + 1)],
                        _ap(wvT_t[:], ci * DIM + ct * 128, [[1, 128]]),
                        xb_t[:, ci, 1024 * c + 512 * j:1024 * c + 512 * (j + 1)],
                        start=(ci == 0), stop=(ci == CT - 1))
            dst = vpad
        else:            # k half
            ct = co
            for j in range(2):
                nc.tensor.matmul(
                    ps[:, 512 * j:512 * (j + 1)],
                    _ap(wkT_t[:], ct * 128, [[DIM, 2], [1, 128]]),
                    _ap(x8f, 1024 * c + 512 * j, [[NPIX, 2], [1, 512]]),
                    start=True, stop=False, perf_mode=DR)
                nc.tensor.matmul(
                    ps[:, 512 * j:512 * (j + 1)],
                    _ap(wkT_t[:], 2 * DIM + ct * 128, [[1, 128]]),
                    x8_t[:, 2, 1024 * c + 512 * j:1024 * c + 512 * (j + 1)],
                    start=False, stop=True)
            dst = kpad
        # evac in parallel halves (scalar + vector): halves the ps ring
        # latency so the PE never waits on a single 1024-wide copy
        psv = ps[:].rearrange("p (a b) -> p a b", a=16)
        e0, e1 = ((nc.scalar.copy, nc.vector.tensor_copy) if (co + c) % 2 == 0
                  else (nc.vector.tensor_copy, nc.scalar.copy))
        e0(dst[:, ct, 1 + 16 * c:9 + 16 * c, 1:65], psv[:, 0:8])
        e1(dst[:, ct, 9 + 16 * c:17 + 16 * c, 1:65], psv[:, 8:16])

    for c in range(4):
        for co in (3, 4, 5):
            a_block(co, c)
    for c in range(4):
        for co in (0, 1, 2):
            a_block(co, c)

    # ---------------- phase B2: k depthwise+pool on PE (fp8 diag DR) ------
    # B3 (k norm, scale washes out) interleaved per ct.
    kp_t = [big.tile([128, NPIX2], BF, tag=f"kp{ct}", name=f"kp{ct}") for ct in range(CT)]
    for ct in range(CT):
        ps = ps_big.tile([128, 1024], F32, tag="ps_big", name="ps_big")
        for i0 in (0, 16):          # output row halves (512 px each)
            for ux in range(4):
                for pp in range(2):  # uy pairs (0,1), (2,3)
                    nc.tensor.matmul(
                        ps[:, 32 * i0:32 * i0 + 512],
                        _ap(dgk_t[:], ct * 2048 + ux * 512 + pp * 256,
                            [[128, 2], [1, 128]]),
                        _ap(kpadf, ct * CB + (2 * i0 + 2 * pp) * RW + ux,
                            [[RW, 2], [2 * RW, 16], [2, 32]]),
                        start=(ux == 0 and pp == 0),
                        stop=(ux == 3 and pp == 1), perf_mode=DR)
        sq = wrk.tile([128, NPIX2], BF, tag="sqk", name="sqk")
        nrm2 = wrk.tile([128, 1], F32, tag="nrm2k", name="nrm2k")
        nc.scalar.activation(sq[:], ps[:], AF.Square, bias=zero_col[:],
                             accum_out=nrm2[:])
        nrm = wrk.tile([128, 1], F32, tag="nrmk", name="nrmk")
        nc.scalar.activation(nrm[:], nrm2[:], AF.Sqrt, bias=eps_col[:])
        inv = wrk.tile([128, 1], F32, tag="invk", name="invk")
        nc.vector.reciprocal(inv[:], nrm[:])
        nc.vector.tensor_scalar_mul(kp_t[ct][:], ps[:], inv[:])
        if "kpn" in dbg_d:
            nc.sync.dma_start(dbg_d["kpn"].ap()[128 * ct:128 * (ct + 1), :], kp_t[ct][:])

    # ---------------- phase B4: kpT via PE transpose (merged evac) --------
    # head-padded layout [128, 8, 64]: head h in cols 64h..64h+47, pad zeroed
    # so head PAIRS sit at 32-aligned partition bases after QK.
    kpT = [big.tile([128, HEADS, 64], BF, tag=f"kpT{pt}", name=f"kpT{pt}") for pt in range(8)]
    q3T = [big.tile([128, HEADS, 64], BF, tag=f"q3T{pt}", name=f"q3T{pt}") for pt in range(8)]
    for pt in range(8):
        nc.vector.memset(kpT[pt][:, :, 48:64], 0.0)
        nc.vector.memset(q3T[pt][:, :, 48:64], 0.0)
    for pt in range(8):
        pst = ps_tr.tile([128, DIM], BF, tag="ps_tr", name="ps_tr")
        for ct in range(CT):
            nc.tensor.transpose(pst[:, 128 * ct:128 * (ct + 1)],
                                kp_t[ct][:, 128 * pt:128 * (pt + 1)], idn_t[:])
        eng = (nc.vector.tensor_copy, nc.scalar.copy)[pt % 2]
        eng(kpT[pt][:, :, 0:48], pst[:].rearrange("p (a b) -> p a b", a=HEADS))

    # ---------------- phase E2: q3T via PE transpose (merged evac) --------
    for pt in range(8):
        pst = ps_tr.tile([128, DIM], BF, tag="ps_tr", name="ps_tr")
        for ct in range(CT):
            nc.tensor.transpose(pst[:, 128 * ct:128 * (ct + 1)],
                                q3_t[ct][:, 128 * pt:128 * (pt + 1)], idn_t[:])
        eng = (nc.vector.tensor_copy, nc.scalar.copy)[pt % 2]
        eng(q3T[pt][:, :, 0:48], pst[:].rearrange("p (a b) -> p a b", a=HEADS))

    # ---------------- phase B1: v depthwise, split across engines ---------
    # PE: 6 taps as bf16 diag matmuls; taps (1,0),(1,1),(1,2) run on the
    # scalar+vector engines and fold into the PSUM evacuation.
    v_dw3 = big.tile([128, CT, NPIX], BF, tag="v_dw3", name="v_dw3")
    v_dwf = v_dw3[:].rearrange("p a b -> p (a b)")
    b1_tmp = {}

    def b1_build(i):
        ct, c = b1_items[i]
        r0 = 8 * c
        w = lambda t9: w3v_t[:, 9 * ct + t9:9 * ct + t9 + 1]
        tmp = wrk.tile([128, 8, 64], BF, tag=f"b1t{(ct * 8 + c) % 3}", name="b1tmp")
        nc.scalar.mul(tmp[:], vpad[:, ct, 1 + r0:9 + r0, 0:64], w(3))
        nc.vector.scalar_tensor_tensor(
            out=tmp[:], in0=vpad[:, ct, 1 + r0:9 + r0, 1:65],
            scalar=w(4), in1=tmp[:], op0=OP.mult, op1=OP.add)
        nc.vector.scalar_tensor_tensor(
            out=tmp[:], in0=vpad[:, ct, 1 + r0:9 + r0, 2:66],
            scalar=w(5), in1=tmp[:], op0=OP.mult, op1=OP.add)
        b1_tmp[(ct, c)] = tmp

    b1_items = [(ct, 2 * g + jj) for g in range(4) for jj in range(2) for ct in range(CT)]
    b1_done = 0

    def b1_block(i):
        ct, c = b1_items[i]
        r0 = 8 * c
        pe_taps = (0, 1, 2, 6, 7, 8)
        ps = ps_sm.tile([128, 512], F32, tag="ps_sm", name="ps_sm")
        for k, t9 in enumerate(pe_taps):
            dy, dx = t9 // 3, t9 % 3
            nc.tensor.matmul(
                ps[:],
                dgv_t[:, ct, t9, :],
                _ap(vpadf, ct * CB + (r0 + dy) * RW + dx, [[RW, 8], [1, 64]]),
                start=(k == 0), stop=(k == len(pe_taps) - 1))
        if i + 1 < len(b1_items):
            b1_build(i + 1)
        tmp = b1_tmp.pop((ct, c))
        nc.vector.scalar_tensor_tensor(
            out=v_dw3[:, ct, 512 * c:512 * (c + 1)],
            in0=ps[:], scalar=1.0,
            in1=tmp[:].rearrange("p a b -> p (a b)"),
            op0=OP.mult, op1=OP.add)

    # ---------------- phase F: QK attn (head pairs) + softmax + M ---------
    mst3 = big.tile([128, CT, DIM], BF, tag="mst3", name="mst3")
    mstf = mst3[:].rearrange("p a b -> p (a b)")
    att_n = []
    b1_build(0)
    for hp in range(4):
        pa = ps_tr.tile([128, 128], F32, tag="ps_tr", name="ps_qk")
        for pt in range(8):
            nc.tensor.matmul(
                pa[:],
                q3T[pt][:, 2 * hp:2 * hp + 2, :].rearrange("p a b -> p (a b)"),
                kpT[pt][:, 2 * hp:2 * hp + 2, :].rearrange("p a b -> p (a b)"),
                start=(pt == 0), stop=(pt == 7))
        # one B1 block between QK pairs keeps the PE fed during softmax
        b1_block(b1_done)
        b1_done += 1
        ae = wrk.tile([128, 128], BF, tag=f"ae{hp % 2}", name=f"ae{hp % 2}")
        zs = wrk.tile([128, 1], F32, tag="zs", name="zs")
        nc.scalar.activation(ae[0:48, 0:48], pa[0:48, 0:48], AF.Exp,
                             bias=zero_col[0:48], accum_out=zs[0:48])
        nc.scalar.activation(ae[64:112, 64:112], pa[64:112, 64:112], AF.Exp,
                             bias=zero_col[0:48], accum_out=zs[64:112])
        zi = wrk.tile([128, 1], F32, tag="zi", name="zi")
        nc.vector.reciprocal(zi[0:48], zs[0:48])
        nc.vector.reciprocal(zi[64:112], zs[64:112])
        an = wrk.tile([128, 128], BF, tag=f"an{hp}", name=f"an{hp}")
        nc.vector.memset(an[:], 0.0)
        nc.vector.tensor_scalar_mul(an[0:48, 0:48], ae[0:48, 0:48], zi[0:48])
        nc.vector.tensor_scalar_mul(an[64:112, 64:112], ae[64:112, 64:112], zi[64:112])
        att_n.append(an)
        if "att" in dbg_d:
            nc.sync.dma_start(dbg_d["att"].ap()[96 * hp:96 * hp + 48, :], an[0:48, 0:48])
            nc.sync.dma_start(dbg_d["att"].ap()[96 * hp + 48:96 * (hp + 1), :], an[64:112, 64:112])
    for hp in range(4):
        an = att_n[hp]
        pm = ps_tr.tile([128, DIM], F32, tag="ps_tr", name="ps_pm")
        nc.tensor.matmul(pm[:], an[:], wpT_t[:, hp, :], start=True, stop=True)
        stg = wrk.tile([128, DIM], BF, tag=f"stg{hp % 2}", name=f"stg{hp % 2}")
        nc.scalar.copy(stg[:], pm[:])
        for half in range(2):
            g0 = HC * (2 * hp + half)
            t0, o0 = divmod(g0, 128)
            n0 = min(128 - o0, HC)
            s0 = 64 * half
            deng = nc.sync if (hp + half) % 2 == 0 else nc.gpsimd
            deng.dma_start(mst3[o0:o0 + n0, t0, :], stg[s0:s0 + n0, :])
            if n0 < HC:
                deng.dma_start(mst3[0:HC - n0, t0 + 1, :], stg[s0 + n0:s0 + HC, :])
    if "mst" in dbg_d:
        nc.sync.dma_start(dbg_d["mst"].ap(), mst3[:].rearrange("p a b -> p (a b)"))

    # ---------------- phases B1 + H interleaved by pixel group ------------
    for g in range(4):
        while b1_done < 6 * (g + 1):
            b1_block(b1_done)
            b1_done += 1
        for ob in range(CT):
            ot = osb.tile([128, 1024], BF, tag="osb", name="osb")
            ps = ps_big.tile([128, 1024], F32, tag="ps_big", name="ps_big")
            for jj in range(2):
                for ctd in range(CT):
                    nc.tensor.matmul(
                        ps[:, 512 * jj:512 * (jj + 1)],
                        _ap(mstf, ctd * DIM + ob * 128, [[1, 128]]),
                        _ap(v_dwf, ctd * NPIX + 1024 * g + 512 * jj, [[1, 512]]),
                        start=(ctd == 0), stop=(ctd == CT - 1))
            if g == 3:
                # final group: split evac + DMA across engines to cut the tail
                nc.scalar.copy(ot[:, 0:512], ps[:, 0:512])
                nc.vector.tensor_copy(ot[:, 512:1024], ps[:, 512:1024])
                nc.sync.dma_start(out_d.ap()[128 * ob:128 * (ob + 1),
                                             1024 * g:1024 * g + 512], ot[:, 0:512])
                nc.gpsimd.dma_start(out_d.ap()[128 * ob:128 * (ob + 1),
                                               1024 * g + 512:1024 * (g + 1)],
                                    ot[:, 512:1024])
            else:
                nc.scalar.copy(ot[:], ps[:])
                deng = nc.sync if (g + ob) % 2 == 0 else nc.gpsimd
                deng.dma_start(out_d.ap()[128 * ob:128 * (ob + 1),
                                          1024 * g:1024 * (g + 1)], ot[:])
    if "vdw" in dbg_d:
        for ct in range(CT):
            nc.sync.dma_start(dbg_d["vdw"].ap()[128 * ct:128 * (ct + 1), :],
                              v_dw3[:, ct, :])
    ctx.close()


# ======================= host-side wrapper =======================

def _f8(a):
    return np.clip(a, -240.0, 240.0).astype(F8_NP)


def _prep_shared(w_kv, w_kv_dw, w_q, w_q_dw, w_proj, temperature):
    """Shared (replicated) weight preprocessing on host."""
    w_kv = np.asarray(w_kv, np.float32)[:, :, 0, 0]          # [768, 384]
    w_kv_dw = np.asarray(w_kv_dw, np.float32)[:, 0]          # [768, 3, 3]
    w_q = np.asarray(w_q, np.float32)[:, :, 0, 0]            # [384, 384]
    w_q_dw = np.asarray(w_q_dw, np.float32)                  # [384, 384, 3, 3]
    w_proj = np.asarray(w_proj, np.float32)[:, :, 0, 0]      # [384, 384]
    temperature = np.asarray(temperature, np.float32).reshape(HEADS)

    # wkT3[ki, ct, co] = w_kv[co, ct*128+ki] * S_A  (k half, fp8)
    wkT3 = np.transpose(
        (w_kv[:DIM] * S_A).reshape(DIM, CT, 128), (2, 1, 0)).reshape(128, -1)
    wvT3 = np.transpose(
        w_kv[DIM:].reshape(DIM, CT, 128), (2, 1, 0)).reshape(128, -1)

    # fused q weights: W2[co, ci, t] = sum_m w_q_dw[co, m, t] * w_q[m, ci]
    W2 = np.einsum("omt,mi->oit",
                   w_q_dw.reshape(DIM, DIM, 9).astype(np.float64),
                   w_q.astype(np.float64)).astype(np.float32)
    s2 = 2.0 ** np.floor(np.log2(200.0 / max(np.abs(W2).max(), 1e-30)))
    # co-major layout [ki, co_t, ci_t, t, cw] so D's co-chunks DMA separately
    w2T = np.transpose((W2 * s2).reshape(CT, 128, CT, 128, 9),
                       (3, 0, 2, 4, 1)).reshape(128, -1)

    w3v = w_kv_dw[DIM:].reshape(DIM, 9)                      # [384, 9] natural
    # fold 2x2 mean pool into k-half depthwise -> 4x4 stride-2 taps
    w3k = w_kv_dw[:DIM]
    w4k = np.zeros((DIM, 4, 4), np.float32)
    for uy in range(4):
        for ux in range(4):
            acc = np.zeros(DIM, np.float32)
            for dy in range(2):
                for dx in range(2):
                    ky, kx = uy - dy, ux - dx
                    if 0 <= ky < 3 and 0 <= kx < 3:
                        acc += w3k[:, ky, kx]
            w4k[:, uy, ux] = 0.25 * acc * S_DW
    w3vc = np.transpose(w3v.reshape(CT, 128, 9), (1, 0, 2)).reshape(128, -1)
    ii = np.arange(128)
    w3v_t = w3v.reshape(CT, 128, 9)
    w4k_t = w4k.reshape(CT, 128, 4, 4)
    dgv = np.zeros((128, CT, 9, 128), np.float32)
    dgk = np.zeros((128, CT, 2048), np.float32)
    for ct in range(CT):
        for t9 in range(9):
            dgv[ii, ct, t9, ii] = w3v_t[ct, :, t9]
        for ux in range(4):
            for pp in range(2):
                dgk[ii, ct, ux * 512 + pp * 256 + ii] = w4k_t[ct, :, 2 * pp, ux]
                dgk[ii, ct, ux * 512 + pp * 256 + 128 + ii] = w4k_t[ct, :, 2 * pp + 1, ux]

    # wpP[64*half + ki, hp, o] = w_proj[o, 48*(2*hp + half) + ki], zero pads
    wpP = np.zeros((128, 4, DIM), np.float32)
    wpt = w_proj.T.reshape(4, 2, HC, DIM)          # [hp, half, ki, o]
    wpP[0:48] = np.transpose(wpt[:, 0], (1, 0, 2))
    wpP[64:112] = np.transpose(wpt[:, 1], (1, 0, 2))
    wpP = wpP.reshape(128, -1)
    tempc = np.repeat(temperature, HC).reshape(CT, 128).T.copy()  # [128, CT]
    idn = np.eye(128, dtype=BF_NP)
    return dict(wkT3=_f8(wkT3), wvT3=wvT3.astype(BF_NP),
                w2T=_f8(w2T), w3vc=w3vc.astype(np.float32),
                dgv=dgv.reshape(128, -1).astype(BF_NP),
                dgk=_f8(dgk.reshape(128, -1)),
                wpP=wpP.astype(BF_NP), tempc=tempc.astype(np.float32),
                idn=idn)


_NC_CACHE = {}


def _get_nc(dbg=False):
    key = bool(dbg)
    if key not in _NC_CACHE:
        _NC_CACHE[key] = build_program(dbg=key)
    return _NC_CACHE[key]


def make_in_maps(x, y, shared):
    x = np.asarray(x, np.float32)
    y = np.asarray(y, np.float32)
    B = x.shape[0]
    in_maps = []
    for b in range(B):
        m = dict(shared)
        # xb3[ki, ci, p] = x[b, ci*128+ki, p]
        xt = np.transpose(x[b].reshape(CT, 128, NPIX), (1, 0, 2)).reshape(128, -1)
        m["xb3"] = xt.astype(BF_NP)
        m["x8"] = _f8(xt)
        # y3p: host-padded fp8 [ki, ct, PW2, RW2]
        yp = np.zeros((128, CT, PW2, RW2), np.float32)
        yp[:, :, 1:1 + H2, 1:1 + H2] = np.transpose(
            y[b].reshape(CT, 128, H2, H2), (1, 0, 2, 3))
        m["y3p"] = _f8(yp.reshape(128, -1))
        in_maps.append(m)
    return in_maps


def kernel(x, y, w_kv, w_kv_dw, w_q, w_q_dw, w_proj, temperature):
    nc = _get_nc(dbg=False)
    shared = _prep_shared(w_kv, w_kv_dw, w_q, w_q_dw, w_proj, temperature)
    in_maps = make_in_maps(x, y, shared)
    res = run_bass_kernel_spmd(nc, in_maps, core_ids=list(range(len(in_maps))))
    out = np.stack([r["out"].astype(np.float32).reshape(DIM, H, H)
                    for r in res.results])
    return out


# revision 62
# speedup vs baseline: 1.0289x; 1.0289x over previous
"""Trainium2 Bass kernel for nn_Attention_49074296324413.

Data-parallel over batch: core b handles batch element b of
  kv = dw3x3(conv1x1(x, w_kv)); k, v = split(kv); k = avgpool2x2(k)
  q  = conv3x3(conv1x1(y, w_q))
  out = conv1x1(softmax(norm(q) @ norm(k).T * temp) @ v, w_proj)

Structure (156.5us baseline -> ~136us):
 - q path: the 1x1 conv is folded into the full 3x3 conv host-side
   (W2[co,ci,t] = sum_m wqdw[co,m,t] wq[m,ci], exact) and the fused conv
   runs FIRST: it needs only ~1.9MB of DMA (y3p host-padded + one
   co-major w2T chunk), so the PE computes while the 4.5MB x streams
   land.  ~8 warm-up matmuls before it flip the HAM clock-gate
   (PE idles cold at 1.2GHz otherwise) during the DMA-issue window.
 - q/k path entirely fp8(e4m3) with DoubleRow (2 contraction blocks per
   pass; pair strides kept %16): quantization noise is washed by the
   per-channel L2 norms + softmax.  k depthwise+pool is folded to a
   4x4-stride-2 conv run as diagonal DR matmuls (8 passes for 16 taps).
 - v path stays bf16 (its error passes straight to the output).
   v-depthwise: 6 taps as PE diag matmuls; taps (1,0),(1,1),(1,2) on
   scalar+vector, folded into the PSUM evacuation (the Tile scheduler
   hoists these builds into earlier idle slots).
 - attn@v and the projection are fused: out = Mst.T @ v_dw with
   M stacked per head from attn_h.T @ w_proj slices.
 - QK/M run on head PAIRS in a head-padded transposed layout
   ([128, 8 heads, 64] with heads at 64-col slots so pair blocks sit at
   32-aligned partitions): 32 QK matmuls + 4 pair M matmuls.
 - PSUM tiles are [128,1024] with single merged evacuations (PSUM f32
   reads run at 1x on the DVE, so fewer/larger evacs matter); transposes
   evacuate 3-at-a-time as [128,384] bf16; the final output group splits
   its evac+DMA across engines to shorten the tail.

Run-to-run variance note: the chip drops the PE to ~2.0GHz under
sustained power draw (P0), which adds ~15-20us on unlucky runs.
"""
import numpy as np
import ml_dtypes

import concourse.bass as bass
import concourse.tile as tile
from concourse import bacc, mybir
from concourse.ap import AP
from concourse.bass_utils import run_bass_kernel_spmd

dt = mybir.dt
BF = dt.bfloat16
F8 = dt.float8e4
F32 = dt.float32
AF = mybir.ActivationFunctionType
OP = mybir.AluOpType
DR = mybir.MatmulPerfMode.DoubleRow

DIM = 384
HEADS = 8
HC = DIM // HEADS          # 48 channels per head
CT = DIM // 128            # 3 channel tiles
H = 64                     # x spatial
NPIX = H * H               # 4096
H2 = 32                    # y spatial
NPIX2 = H2 * H2            # 1024
PW = 66                    # padded rows for 64-grid
RW = 80                    # padded row pitch (elems) for 64-grid
PW2 = 34                   # padded rows for 32-grid
RW2 = 48                   # padded row pitch for 32-grid
CB = PW * RW               # 5280 elems per channel-tile block (64-grid)
CB2 = PW2 * RW2            # 1632 elems per channel-tile block (32-grid)

BF_NP = ml_dtypes.bfloat16
F8_NP = ml_dtypes.float8_e4m3

S_A = 2.0 ** 6     # w_k scale (washed by k-norm)
S_DW = 2.0 ** 11   # w4k (k depthwise+pool) scale (washed by k-norm)


def _ap(base: AP, off: int, dims):
    """Custom strided AP into a tile's free space: dims = [[stride, n], ...]."""
    return AP(tensor=base.tensor, offset=base.offset + off,
              ap=[[base.ap[0][0], base.ap[0][1]]] + [list(d) for d in dims])


def build_program(dbg: bool = False):
    nc = bacc.Bacc("TRN2", target_bir_lowering=False, debug=False)

    xb_d = nc.dram_tensor("xb3", (128, CT * NPIX), BF, kind="ExternalInput")
    x8_d = nc.dram_tensor("x8", (128, CT * NPIX), F8, kind="ExternalInput")
    y3p_d = nc.dram_tensor("y3p", (128, CT * CB2), F8, kind="ExternalInput")
    wkT_d = nc.dram_tensor("wkT3", (128, CT * DIM), F8, kind="ExternalInput")
    wvT_d = nc.dram_tensor("wvT3", (128, CT * DIM), BF, kind="ExternalInput")
    w2T_d = nc.dram_tensor("w2T", (128, CT * 9 * DIM), F8, kind="ExternalInput")
    w3v_d = nc.dram_tensor("w3vc", (128, CT * 9), F32, kind="ExternalInput")
    dgv_d = nc.dram_tensor("dgv", (128, CT * 9 * 128), BF, kind="ExternalInput")
    dgk_d = nc.dram_tensor("dgk", (128, CT * 2048), F8, kind="ExternalInput")
    wpT_d = nc.dram_tensor("wpP", (128, 4 * DIM), BF, kind="ExternalInput")
    temp_d = nc.dram_tensor("tempc", (128, CT), F32, kind="ExternalInput")
    idn_d = nc.dram_tensor("idn", (128, 128), BF, kind="ExternalInput")

    out_d = nc.dram_tensor("out", (DIM, NPIX), BF, kind="ExternalOutput")
    dbg_d = {}
    if dbg:
        dbg_d["vdw"] = nc.dram_tensor("dbg_vdw", (DIM, NPIX), BF, kind="ExternalOutput")
        dbg_d["kpn"] = nc.dram_tensor("dbg_kpn", (DIM, NPIX2), BF, kind="ExternalOutput")
        dbg_d["q3n"] = nc.dram_tensor("dbg_q3n", (DIM, NPIX2), BF, kind="ExternalOutput")
        dbg_d["att"] = nc.dram_tensor("dbg_att", (HEADS * HC, HC), BF, kind="ExternalOutput")
        dbg_d["mst"] = nc.dram_tensor("dbg_mst", (128, CT * DIM), BF, kind="ExternalOutput")

    with tile.TileContext(nc) as tc:
        _emit(nc, tc, xb_d, x8_d, y3p_d, wkT_d, wvT_d, w2T_d, w3v_d,
              dgv_d, dgk_d, wpT_d, temp_d, idn_d, out_d, dbg_d)
    nc.compile()
    return nc


def _emit(nc, tc, xb_d, x8_d, y3p_d, wkT_d, wvT_d, w2T_d, w3v_d,
          dgv_d, dgk_d, wpT_d, temp_d, idn_d, out_d, dbg_d):
    from contextlib import ExitStack
    ctx = ExitStack()

    cst = ctx.enter_context(tc.tile_pool(name="cst", bufs=1))
    big = ctx.enter_context(tc.tile_pool(name="big", bufs=1))
    wrk = ctx.enter_context(tc.tile_pool(name="wrk", bufs=2))
    osb = ctx.enter_context(tc.tile_pool(name="osb", bufs=4))
    ps_big = ctx.enter_context(tc.tile_pool(name="ps_big", bufs=2, space="PSUM"))
    ps_sm = ctx.enter_context(tc.tile_pool(name="ps_sm", bufs=2, space="PSUM"))
    ps_tr = ctx.enter_context(tc.tile_pool(name="ps_tr", bufs=2, space="PSUM"))

    # ---------------- PE warm-up (HAM un-throttle during DMA window) ------
    warm = cst.tile([128, 512], BF, tag="warm", name="warm")
    nc.vector.memset(warm[:], 0.25)

    # ---------------- input DMAs, priority-ordered per queue --------------
    # D (fused q conv) runs FIRST while the big xb/x8 streams land, so the
    # sync queue leads with w2T co-chunks and scalar with y3p.
    w2T_t = cst.tile([128, CT * 9 * DIM], F8, tag="w2T", name="w2T")
    w2v = w2T_t[:].rearrange("p (a b) -> p a b", a=CT)
    w2d = w2T_d.ap().rearrange("p (a b) -> p a b", a=CT)
    nc.sync.dma_start(w2v[:, 0], w2d[:, 0])
    y3p_t = cst.tile([128, CT, PW2, RW2], F8, tag="y3p", name="y3p")
    nc.scalar.dma_start(y3p_t[:].rearrange("p a b c -> p (a b c)"), y3p_d.ap())
    xb_t = cst.tile([128, CT, NPIX], BF, tag="xb3", name="xb3")
    xbv = xb_d.ap().rearrange("p (a b) -> p a b", a=CT)
    # hold gpsimd's first (late-needed) xb issue until w2T chunk 0 has
    # landed, so D's critical-path DMA gets the full HBM bandwidth
    gate = cst.tile([128, 1], F8, tag="gate", name="gate")
    nc.gpsimd.tensor_copy(gate[:], w2T_t[:, 0:1])
    nc.gpsimd.dma_start(xb_t[:, :, 1024:2048], xbv[:, :, 1024:2048])
    nc.sync.dma_start(w2v[:, 1], w2d[:, 1])
    nc.sync.dma_start(w2v[:, 2], w2d[:, 2])
    dgv_t = cst.tile([128, CT, 9, 128], BF, tag="dgv", name="dgv")
    nc.sync.dma_start(dgv_t[:].rearrange("p a b c -> p (a b c)"), dgv_d.ap())
    tempc_t = cst.tile([128, CT], F32, tag="tempc", name="tempc")
    nc.scalar.dma_start(tempc_t[:], temp_d.ap())
    wvT_t = cst.tile([128, CT * DIM], BF, tag="wvT", name="wvT")
    nc.scalar.dma_start(wvT_t[:], wvT_d.ap())
    nc.sync.dma_start(xb_t[:, :, 0:1024], xbv[:, :, 0:1024])
    nc.gpsimd.dma_start(xb_t[:, :, 3072:4096], xbv[:, :, 3072:4096])
    wkT_t = cst.tile([128, CT * DIM], F8, tag="wkT", name="wkT")
    nc.scalar.dma_start(wkT_t[:], wkT_d.ap())
    nc.sync.dma_start(xb_t[:, :, 2048:3072], xbv[:, :, 2048:3072])
    w3v_t = cst.tile([128, CT * 9], F32, tag="w3vc", name="w3vc")
    nc.scalar.dma_start(w3v_t[:], w3v_d.ap())
    dgk_t = cst.tile([128, CT * 2048], F8, tag="dgk", name="dgk")
    nc.scalar.dma_start(dgk_t[:], dgk_d.ap())
    idn_t = cst.tile([128, 128], BF, tag="idn", name="idn")
    nc.scalar.dma_start(idn_t[:], idn_d.ap())
    wpT_t = cst.tile([128, 4, DIM], BF, tag="wpP", name="wpP")
    nc.scalar.dma_start(wpT_t[:].rearrange("p a b -> p (a b)"), wpT_d.ap())

    eps_col = cst.tile([128, 1], F32, tag="eps_col", name="eps_col")
    nc.vector.memset(eps_col[:], 1e-24)
    zero_col = cst.tile([128, 1], F32, tag="zero_col", name="zero_col")
    nc.vector.memset(zero_col[:], 0.0)

    # warm-up matmuls at the cold clock: flips the HAM SHORT window and
    # fills the PE until D's first DMA dependencies land (~13us).
    for _ in range(10):
        psw = ps_sm.tile([128, 512], F32, tag="ps_sm", name="ps_sm")
        nc.tensor.matmul(psw[:], warm[:, 0:128], warm[:], start=True, stop=True)

    # ---------------- padded image buffers (zero borders) ----------------
    kpad = big.tile([128, CT, PW, RW], F8, tag="kpad", name="kpad")
    vpad = big.tile([128, CT, PW, RW], BF, tag="vpad", name="vpad")
    for ct in range(CT):
        for t, pw in ((kpad, PW), (vpad, PW)):
            eng = nc.vector if ct % 2 == 0 else nc.gpsimd
            eng.memset(t[:, ct, 0, :], 0.0)
            eng.memset(t[:, ct, pw - 1, :], 0.0)
            eng.memset(t[:, ct, 1:pw - 1, 0:1], 0.0)
            eng.memset(t[:, ct, 1:pw - 1, pw - 1:pw], 0.0)
    kpadf = kpad[:].rearrange("p a b c -> p (a b c)")
    vpadf = vpad[:].rearrange("p a b c -> p (a b c)")
    y3pf = y3p_t[:].rearrange("p a b c -> p (a b c)")

    # ---------------- x8 = fp8(x) via DMA (behind the xb chunks) ----------
    x8_t = cst.tile([128, CT, NPIX], F8, tag="x8", name="x8")
    x8f = x8_t[:].rearrange("p a b -> p (a b)")
    x8v = x8_d.ap().rearrange("p (a b) -> p a b", a=CT)
    nc.gpsimd.dma_start(x8_t[:, :, 0:2048], x8v[:, :, 0:2048])
    nc.gpsimd.dma_start(x8_t[:, :, 2048:4096], x8v[:, :, 2048:4096])

    # ---------------- phase D: q3 = fused conv3x3(conv1x1(y)) (fp8 DR) ----
    # Runs FIRST: needs only y3p + one w2T co-chunk, filling the PE while
    # the xb/x8 streams land.  Contraction blocks b=(ci,dy) lex-ordered;
    # pairs share dx.  E (q norm + temperature) follows per co.
    q3_t = [big.tile([128, NPIX2], BF, tag=f"q3{ct}", name=f"q3{ct}") for ct in range(CT)]
    blocks = [(ci, dy) for ci in range(CT) for dy in range(3)]
    for co in range(CT):
        ps = ps_big.tile([128, 1024], F32, tag="ps_big", name="ps_big")
        for j in range(2):
            for dx in range(3):
                for p in range(4):
                    ci0, dy0 = blocks[2 * p]
                    ci1, dy1 = blocks[2 * p + 1]
                    m0 = ci0 * CB2 + (16 * j + dy0) * RW2 + dx
                    dm = (ci1 - ci0) * CB2 + (dy1 - dy0) * RW2
                    w0 = co * 9 * DIM + (ci0 * 9 + 3 * dy0 + dx) * 128
                    nc.tensor.matmul(
                        ps[:, 512 * j:512 * (j + 1)],
                        _ap(w2T_t[:], w0, [[3 * 128, 2], [1, 128]]),
                        _ap(y3pf, m0, [[dm, 2], [RW2, 16], [1, 32]]),
                        start=(dx == 0 and p == 0), stop=False, perf_mode=DR)
                m8 = 2 * CB2 + (16 * j + 2) * RW2 + dx
                w8 = co * 9 * DIM + (2 * 9 + 6 + dx) * 128
                nc.tensor.matmul(
                    ps[:, 512 * j:512 * (j + 1)],
                    _ap(w2T_t[:], w8, [[1, 128]]),
                    _ap(y3pf, m8, [[RW2, 16], [1, 32]]),
                    start=False, stop=(dx == 2))
        # phase E: q norm + temperature for this co, fused with the PSUM
        # evacuation (Square reads PSUM; the normalize mul writes q3_t).
        sq = wrk.tile([128, NPIX2], BF, tag="sqq", name="sqq")
        nrm2 = wrk.tile([128, 1], F32, tag="nrm2q", name="nrm2q")
        nc.scalar.activation(sq[:], ps[:], AF.Square, bias=zero_col[:],
                             accum_out=nrm2[:])
        nrm = wrk.tile([128, 1], F32, tag="nrmq", name="nrmq")
        nc.scalar.activation(nrm[:], nrm2[:], AF.Sqrt, bias=eps_col[:])
        inv = wrk.tile([128, 1], F32, tag="invq", name="invq")
        nc.vector.reciprocal(inv[:], nrm[:])
        invt = wrk.tile([128, 1], F32, tag="invqt", name="invqt")
        nc.scalar.mul(invt[:], inv[:], tempc_t[:, co:co + 1])
        nc.vector.tensor_scalar_mul(q3_t[co][:], ps[:], invt[:])
        if "q3n" in dbg_d:
            nc.sync.dma_start(dbg_d["q3n"].ap()[128 * co:128 * (co + 1), :], q3_t[co][:])

    # ---------------- phase A: kv1 = W_kv @ x -----------------------------
    # v half bf16 (precision), k half fp8 DR; FD=1024.
    def a_block(co, c):
        ps = ps_big.tile([128, 1024], F32, tag="ps_big", name="ps_big")
        if co >= CT:     # v half
            ct = co - CT
            for j in range(2):
                for ci in range(CT):
                    nc.tensor.matmul(
                        ps[:, 512 * j:512 * (j + 1)],
                        _ap(wvT_t[:], ci * DIM + ct * 128, [[1, 128]]),
                        xb_t[:, ci, 1024 * c + 512 * j:1024 * c + 512 * (j + 1)],
                        start=(ci == 0), stop=(ci == CT - 1))
            dst = vpad
        else:            # k half
            ct = co
            for j in range(2):
                nc.tensor.matmul(
                    ps[:, 512 * j:512 * (j + 1)],
                    _ap(wkT_t[:], ct * 128, [[DIM, 2], [1, 128]]),
                    _ap(x8f, 1024 * c + 512 * j, [[NPIX, 2], [1, 512]]),
                    start=True, stop=False, perf_mode=DR)
                nc.tensor.matmul(
                    ps[:, 512 * j:512 * (j + 1)],
                    _ap(wkT_t[:], 2 * DIM + ct * 128, [[1, 128]]),
                    x8_t[:, 2, 1024 * c + 512 * j:1024 * c + 512 * (j + 1)],
                    start=False, stop=True)
            dst = kpad
        eng = nc.scalar.copy if (co + c) % 2 == 0 else nc.vector.tensor_copy
        eng(dst[:, ct, 1 + 16 * c:17 + 16 * c, 1:65],
            ps[:].rearrange("p (a b) -> p a b", a=16))

    for c in range(4):
        for co in (3, 4, 5):
            a_block(co, c)
    for c in range(4):
        for co in (0, 1, 2):
            a_block(co, c)

    # ---------------- phase B2: k depthwise+pool on PE (fp8 diag DR) ------
    # B3 (k norm, scale washes out) interleaved per ct.
    kp_t = [big.tile([128, NPIX2], BF, tag=f"kp{ct}", name=f"kp{ct}") for ct in range(CT)]
    for ct in range(CT):
        ps = ps_big.tile([128, 1024], F32, tag="ps_big", name="ps_big")
        for i0 in (0, 16):          # output row halves (512 px each)
            for ux in range(4):
                for pp in range(2):  # uy pairs (0,1), (2,3)
                    nc.tensor.matmul(
                        ps[:, 32 * i0:32 * i0 + 512],
                        _ap(dgk_t[:], ct * 2048 + ux * 512 + pp * 256,
                            [[128, 2], [1, 128]]),
                        _ap(kpadf, ct * CB + (2 * i0 + 2 * pp) * RW + ux,
                            [[RW, 2], [2 * RW, 16], [2, 32]]),
                        start=(ux == 0 and pp == 0),
                        stop=(ux == 3 and pp == 1), perf_mode=DR)
        sq = wrk.tile([128, NPIX2], BF, tag="sqk", name="sqk")
        nrm2 = wrk.tile([128, 1], F32, tag="nrm2k", name="nrm2k")
        nc.scalar.activation(sq[:], ps[:], AF.Square, bias=zero_col[:],
                             accum_out=nrm2[:])
        nrm = wrk.tile([128, 1], F32, tag="nrmk", name="nrmk")
        nc.scalar.activation(nrm[:], nrm2[:], AF.Sqrt, bias=eps_col[:])
        inv = wrk.tile([128, 1], F32, tag="invk", name="invk")
        nc.vector.reciprocal(inv[:], nrm[:])
        nc.vector.tensor_scalar_mul(kp_t[ct][:], ps[:], inv[:])
        if "kpn" in dbg_d:
            nc.sync.dma_start(dbg_d["kpn"].ap()[128 * ct:128 * (ct + 1), :], kp_t[ct][:])

    # ---------------- phase B4: kpT via PE transpose (merged evac) --------
    # head-padded layout [128, 8, 64]: head h in cols 64h..64h+47, pad zeroed
    # so head PAIRS sit at 32-aligned partition bases after QK.
    kpT = [big.tile([128, HEADS, 64], BF, tag=f"kpT{pt}", name=f"kpT{pt}") for pt in range(8)]
    q3T = [big.tile([128, HEADS, 64], BF, tag=f"q3T{pt}", name=f"q3T{pt}") for pt in range(8)]
    for pt in range(8):
        nc.vector.memset(kpT[pt][:, :, 48:64], 0.0)
        nc.vector.memset(q3T[pt][:, :, 48:64], 0.0)
    for pt in range(8):
        pst = ps_tr.tile([128, DIM], BF, tag="ps_tr", name="ps_tr")
        for ct in range(CT):
            nc.tensor.transpose(pst[:, 128 * ct:128 * (ct + 1)],
                                kp_t[ct][:, 128 * pt:128 * (pt + 1)], idn_t[:])
        eng = (nc.vector.tensor_copy, nc.scalar.copy)[pt % 2]
        eng(kpT[pt][:, :, 0:48], pst[:].rearrange("p (a b) -> p a b", a=HEADS))

    # ---------------- phase E2: q3T via PE transpose (merged evac) --------
    for pt in range(8):
        pst = ps_tr.tile([128, DIM], BF, tag="ps_tr", name="ps_tr")
        for ct in range(CT):
            nc.tensor.transpose(pst[:, 128 * ct:128 * (ct + 1)],
                                q3_t[ct][:, 128 * pt:128 * (pt + 1)], idn_t[:])
        eng = (nc.vector.tensor_copy, nc.scalar.copy)[pt % 2]
        eng(q3T[pt][:, :, 0:48], pst[:].rearrange("p (a b) -> p a b", a=HEADS))

    # ---------------- phase B1: v depthwise, split across engines ---------
    # PE: 6 taps as bf16 diag matmuls; taps (1,0),(1,1),(1,2) run on the
    # scalar+vector engines and fold into the PSUM evacuation.
    v_dw3 = big.tile([128, CT, NPIX], BF, tag="v_dw3", name="v_dw3")
    v_dwf = v_dw3[:].rearrange("p a b -> p (a b)")
    b1_tmp = {}

    def b1_build(i):
        ct, c = b1_items[i]
        r0 = 8 * c
        w = lambda t9: w3v_t[:, 9 * ct + t9:9 * ct + t9 + 1]
        tmp = wrk.tile([128, 8, 64], BF, tag=f"b1t{(ct * 8 + c) % 3}", name="b1tmp")
        nc.scalar.mul(tmp[:], vpad[:, ct, 1 + r0:9 + r0, 0:64], w(3))
        nc.vector.scalar_tensor_tensor(
            out=tmp[:], in0=vpad[:, ct, 1 + r0:9 + r0, 1:65],
            scalar=w(4), in1=tmp[:], op0=OP.mult, op1=OP.add)
        nc.vector.scalar_tensor_tensor(
            out=tmp[:], in0=vpad[:, ct, 1 + r0:9 + r0, 2:66],
            scalar=w(5), in1=tmp[:], op0=OP.mult, op1=OP.add)
        b1_tmp[(ct, c)] = tmp

    b1_items = [(ct, 2 * g + jj) for g in range(4) for jj in range(2) for ct in range(CT)]
    b1_done = 0

    def b1_block(i):
        ct, c = b1_items[i]
        r0 = 8 * c
        pe_taps = (0, 1, 2, 6, 7, 8)
        ps = ps_sm.tile([128, 512], F32, tag="ps_sm", name="ps_sm")
        for k, t9 in enumerate(pe_taps):
            dy, dx = t9 // 3, t9 % 3
            nc.tensor.matmul(
                ps[:],
                dgv_t[:, ct, t9, :],
                _ap(vpadf, ct * CB + (r0 + dy) * RW + dx, [[RW, 8], [1, 64]]),
                start=(k == 0), stop=(k == len(pe_taps) - 1))
        if i + 1 < len(b1_items):
            b1_build(i + 1)
        tmp = b1_tmp.pop((ct, c))
        nc.vector.scalar_tensor_tensor(
            out=v_dw3[:, ct, 512 * c:512 * (c + 1)],
            in0=ps[:], scalar=1.0,
            in1=tmp[:].rearrange("p a b -> p (a b)"),
            op0=OP.mult, op1=OP.add)

    # ---------------- phase F: QK attn (head pairs) + softmax + M ---------
    mst3 = big.tile([128, CT, DIM], BF, tag="mst3", name="mst3")
    mstf = mst3[:].rearrange("p a b -> p (a b)")
    att_n = []
    b1_build(0)
    for hp in range(4):
        pa = ps_tr.tile([128, 128], F32, tag="ps_tr", name="ps_qk")
        for pt in range(8):
            nc.tensor.matmul(
                pa[:],
                q3T[pt][:, 2 * hp:2 * hp + 2, :].rearrange("p a b -> p (a b)"),
                kpT[pt][:, 2 * hp:2 * hp + 2, :].rearrange("p a b -> p (a b)"),
                start=(pt == 0), stop=(pt == 7))
        # one B1 block between QK pairs keeps the PE fed during softmax
        b1_block(b1_done)
        b1_done += 1
        ae = wrk.tile([128, 128], BF, tag=f"ae{hp % 2}", name=f"ae{hp % 2}")
        zs = wrk.tile([128, 1], F32, tag="zs", name="zs")
        nc.scalar.activation(ae[0:48, 0:48], pa[0:48, 0:48], AF.Exp,
                             bias=zero_col[0:48], accum_out=zs[0:48])
        nc.scalar.activation(ae[64:112, 64:112], pa[64:112, 64:112], AF.Exp,
                             bias=zero_col[0:48], accum_out=zs[64:112])
        zi = wrk.tile([128, 1], F32, tag="zi", name="zi")
        nc.vector.reciprocal(zi[0:48], zs[0:48])
        nc.vector.reciprocal(zi[64:112], zs[64:112])
        an = wrk.tile([128, 128], BF, tag=f"an{hp}", name=f"an{hp}")
        nc.vector.memset(an[:], 0.0)
        nc.vector.tensor_scalar_mul(an[0:48, 0:48], ae[0:48, 0:48], zi[0:48])
        nc.vector.tensor_scalar_mul(an[64:112, 64:112], ae[64:112, 64:112], zi[64:112])
        att_n.append(an)
        if "att" in dbg_d:
            nc.sync.dma_start(dbg_d["att"].ap()[96 * hp:96 * hp + 48, :], an[0:48, 0:48])
            nc.sync.dma_start(dbg_d["att"].ap()[96 * hp + 48:96 * (hp + 1), :], an[64:112, 64:112])
    for hp in range(4):
        an = att_n[hp]
        pm = ps_tr.tile([128, DIM], F32, tag="ps_tr", name="ps_pm")
        nc.tensor.matmul(pm[:], an[:], wpT_t[:, hp, :], start=True, stop=True)
        stg = wrk.tile([128, DIM], BF, tag=f"stg{hp % 2}", name=f"stg{hp % 2}")
        nc.scalar.copy(stg[:], pm[:])
        for half in range(2):
            g0 = HC * (2 * hp + half)
            t0, o0 = divmod(g0, 128)
            n0 = min(128 - o0, HC)
            s0 = 64 * half
            deng = nc.sync if (hp + half) % 2 == 0 else nc.gpsimd
            deng.dma_start(mst3[o0:o0 + n0, t0, :], stg[s0:s0 + n0, :])
            if n0 < HC:
                deng.dma_start(mst3[0:HC - n0, t0 + 1, :], stg[s0 + n0:s0 + HC, :])
    if "mst" in dbg_d:
        nc.sync.dma_start(dbg_d["mst"].ap(), mst3[:].rearrange("p a b -> p (a b)"))

    # ---------------- phases B1 + H interleaved by pixel group ------------
    for g in range(4):
        while b1_done < 6 * (g + 1):
            b1_block(b1_done)
            b1_done += 1
        for ob in range(CT):
            ot = osb.tile([128, 1024], BF, tag="osb", name="osb")
            ps = ps_big.tile([128, 1024], F32, tag="ps_big", name="ps_big")
            for jj in range(2):
                for ctd in range(CT):
                    nc.tensor.matmul(
                        ps[:, 512 * jj:512 * (jj + 1)],
                        _ap(mstf, ctd * DIM + ob * 128, [[1, 128]]),
                        _ap(v_dwf, ctd * NPIX + 1024 * g + 512 * jj, [[1, 512]]),
                        start=(ctd == 0), stop=(ctd == CT - 1))
            if g == 3:
                # final group: split evac + DMA across engines to cut the tail
                nc.scalar.copy(ot[:, 0:512], ps[:, 0:512])
                nc.vector.tensor_copy(ot[:, 512:1024], ps[:, 512:1024])
                nc.sync.dma_start(out_d.ap()[128 * ob:128 * (ob + 1),
                                             1024 * g:1024 * g + 512], ot[:, 0:512])
                nc.gpsimd.dma_start(out_d.ap()[128 * ob:128 * (ob + 1),
                                               1024 * g + 512:1024 * (g + 1)],
                                    ot[:, 512:1024])
            else:
                nc.scalar.copy(ot[:], ps[:])
                deng = nc.sync if (g + ob) % 2 == 0 else nc.gpsimd
                deng.dma_start(out_d.ap()[128 * ob:128 * (ob + 1),
                                          1024 * g:1024 * (g + 1)], ot[:])
    if "vdw" in dbg_d:
        for ct in range(CT):
            nc.sync.dma_start(dbg_d["vdw"].ap()[128 * ct:128 * (ct + 1), :],
                              v_dw3[:, ct, :])
    ctx.close()


# ======================= host-side wrapper =======================

def _f8(a):
    return np.clip(a, -240.0, 240.0).astype(F8_NP)


def _prep_shared(w_kv, w_kv_dw, w_q, w_q_dw, w_proj, temperature):
    """Shared (replicated) weight preprocessing on host."""
    w_kv = np.asarray(w_kv, np.float32)[:, :, 0, 0]          # [768, 384]
    w_kv_dw = np.asarray(w_kv_dw, np.float32)[:, 0]          # [768, 3, 3]
    w_q = np.asarray(w_q, np.float32)[:, :, 0, 0]            # [384, 384]
    w_q_dw = np.asarray(w_q_dw, np.float32)                  # [384, 384, 3, 3]
    w_proj = np.asarray(w_proj, np.float32)[:, :, 0, 0]      # [384, 384]
    temperature = np.asarray(temperature, np.float32).reshape(HEADS)

    # wkT3[ki, ct, co] = w_kv[co, ct*128+ki] * S_A  (k half, fp8)
    wkT3 = np.transpose(
        (w_kv[:DIM] * S_A).reshape(DIM, CT, 128), (2, 1, 0)).reshape(128, -1)
    wvT3 = np.transpose(
        w_kv[DIM:].reshape(DIM, CT, 128), (2, 1, 0)).reshape(128, -1)

    # fused q weights: W2[co, ci, t] = sum_m w_q_dw[co, m, t] * w_q[m, ci]
    W2 = np.einsum("omt,mi->oit",
                   w_q_dw.reshape(DIM, DIM, 9).astype(np.float64),
                   w_q.astype(np.float64)).astype(np.float32)
    s2 = 2.0 ** np.floor(np.log2(200.0 / max(np.abs(W2).max(), 1e-30)))
    # co-major layout [ki, co_t, ci_t, t, cw] so D's co-chunks DMA separately
    w2T = np.transpose((W2 * s2).reshape(CT, 128, CT, 128, 9),
                       (3, 0, 2, 4, 1)).reshape(128, -1)

    w3v = w_kv_dw[DIM:].reshape(DIM, 9)                      # [384, 9] natural
    # fold 2x2 mean pool into k-half depthwise -> 4x4 stride-2 taps
    w3k = w_kv_dw[:DIM]
    w4k = np.zeros((DIM, 4, 4), np.float32)
    for uy in range(4):
        for ux in range(4):
            acc = np.zeros(DIM, np.float32)
            for dy in range(2):
                for dx in range(2):
                    ky, kx = uy - dy, ux - dx
                    if 0 <= ky < 3 and 0 <= kx < 3:
                        acc += w3k[:, ky, kx]
            w4k[:, uy, ux] = 0.25 * acc * S_DW
    w3vc = np.transpose(w3v.reshape(CT, 128, 9), (1, 0, 2)).reshape(128, -1)
    ii = np.arange(128)
    w3v_t = w3v.reshape(CT, 128, 9)
    w4k_t = w4k.reshape(CT, 128, 4, 4)
    dgv = np.zeros((128, CT, 9, 128), np.float32)
    dgk = np.zeros((128, CT, 2048), np.float32)
    for ct in range(CT):
        for t9 in range(9):
            dgv[ii, ct, t9, ii] = w3v_t[ct, :, t9]
        for ux in range(4):
            for pp in range(2):
                dgk[ii, ct, ux * 512 + pp * 256 + ii] = w4k_t[ct, :, 2 * pp, ux]
                dgk[ii, ct, ux * 512 + pp * 256 + 128 + ii] = w4k_t[ct, :, 2 * pp + 1, ux]

    # wpP[64*half + ki, hp, o] = w_proj[o, 48*(2*hp + half) + ki], zero pads
    wpP = np.zeros((128, 4, DIM), np.float32)
    wpt = w_proj.T.reshape(4, 2, HC, DIM)          # [hp, half, ki, o]
    wpP[0:48] = np.transpose(wpt[:, 0], (1, 0, 2))
    wpP[64:112] = np.transpose(wpt[:, 1], (1, 0, 2))
    wpP = wpP.reshape(128, -1)
    tempc = np.repeat(temperature, HC).reshape(CT, 128).T.copy()  # [128, CT]
    idn = np.eye(128, dtype=BF_NP)
    return dict(wkT3=_f8(wkT3), wvT3=wvT3.astype(BF_NP),
                w2T=_f8(w2T), w3vc=w3vc.astype(np.float32),
                dgv=dgv.reshape(128, -1).astype(BF_NP),
                dgk=_f8(dgk.reshape(128, -1)),
                wpP=wpP.astype(BF_NP), tempc=tempc.astype(np.float32),
                idn=idn)


_NC_CACHE = {}


def _get_nc(dbg=False):
    key = bool(dbg)
    if key not in _NC_CACHE:
        _NC_CACHE[key] = build_program(dbg=key)
    return _NC_CACHE[key]


def make_in_maps(x, y, shared):
    x = np.asarray(x, np.float32)
    y = np.asarray(y, np.float32)
    B = x.shape[0]
    in_maps = []
    for b in range(B):
        m = dict(shared)
        # xb3[ki, ci, p] = x[b, ci*128+ki, p]
        xt = np.transpose(x[b].reshape(CT, 128, NPIX), (1, 0, 2)).reshape(128, -1)
        m["xb3"] = xt.astype(BF_NP)
        m["x8"] = _f8(xt)
        # y3p: host-padded fp8 [ki, ct, PW2, RW2]
        yp = np.zeros((128, CT, PW2, RW2), np.float32)
        yp[:, :, 1:1 + H2, 1:1 + H2] = np.transpose(
            y[b].reshape(CT, 128, H2, H2), (1, 0, 2, 3))
        m["y3p"] = _f8(yp.reshape(128, -1))
        in_maps.append(m)
    return in_maps


def kernel(x, y, w_kv, w_kv_dw, w_q, w_q_dw, w_proj, temperature):
    nc = _get_nc(dbg=False)
    shared = _prep_shared(w_kv, w_kv_dw, w_q, w_q_dw, w_proj, temperature)
    in_maps = make_in_maps(x, y, shared)
    res = run_bass_kernel_spmd(nc, in_maps, core_ids=list(range(len(in_maps))))
    out = np.stack([r["out"].astype(np.float32).reshape(DIM, H, H)
                    for r in res.results])
    return out
